# revision 1
# baseline (speedup 1.0000x reference)
"""Trainium2 Bass kernel for patch-attention (nn_Attention_58755152609998).

Computation per example:
  x [64,64,256] -> 8x8 grid of 8x8 patches -> x_fc [N=64 patches, P=64 pos, C=256]
  qkv = x_fc @ w_qkv  [N, P, 768]; per (n): flat [P*768] viewed as [3, h=8, P, d=32]
  per head: q_h/k_h/v_h = contiguous 2048-wide slices of the flat buffer,
  attention over the 64 patch tokens, o flat2 [h*2048] viewed [P, C] contiguously,
  y = o_patch @ w_proj + b_proj, reassembled to [64,64,256].

Sharding: pure data-parallel, batch 32 -> 4 examples on each of 8 cores.

Layout: tokens kept in RASTER order t = row*64 + col = gi*512 + pi*64 + gj*8 + pj
throughout (clean single-stride-partition DMAs).  Since 768 = 6*128, the
128-aligned chunks of the per-patch flat qkv buffer are (p = m//6, cc-chunk
m%6) tiles; with raster order a chunk's 64 patch-tokens sit at strided SBUF
positions pi*64+pj + {gi:512}x{gj:8}, expressed as strided matmul-operand APs.
"""

import numpy as np

B_GLOBAL = 32
N_CORES = 8
B_LOC = B_GLOBAL // N_CORES      # 4 examples per core
C = 256
G = 8          # patch grid
PS = 8         # patch size
N = G * G      # 64 patches (attention tokens)
P = PS * PS    # 64 positions per patch
H = 8          # heads
D = 32         # C // H
SCALE = float((D * P) ** -0.5)


def _build_nc(debug_dump=False, xin_bufs=14, attn_bufs=4, y_bufs=6, bank_bufs=3, small_bufs=1, xt_bufs=1, ot_bufs=1):
    import concourse.bass as bass
    import concourse.bacc as bacc
    import concourse.tile as tile
    from concourse import mybir
    from concourse.masks import make_identity

    fp32 = mybir.dt.float32
    bf16 = mybir.dt.bfloat16

    nc = bacc.Bacc(
        "TRN2",
        target_bir_lowering=False,
        debug=False,
        enable_asserts=False,
        num_devices=N_CORES,
    )

    x_t = nc.dram_tensor("x", [B_LOC, 64, 64, C], fp32, kind="ExternalInput")
    wq_t = nc.dram_tensor("w_qkv", [C, 3 * C], fp32, kind="ExternalInput")
    wp_t = nc.dram_tensor("w_proj", [C, C], fp32, kind="ExternalInput")
    bp_t = nc.dram_tensor("b_proj", [C], fp32, kind="ExternalInput")
    out_t = nc.dram_tensor("out", [B_LOC, 64, 64, C], fp32, kind="ExternalOutput")

    out_flat = out_t.ap().rearrange("b r w c -> b (r w) c")  # [B, 4096, C]

    dbg = {}
    if debug_dump:
        dbg["xt"] = nc.dram_tensor("dbg_xt", [2, 128, P * N], bf16, kind="ExternalOutput")
        dbg["qkvt"] = nc.dram_tensor("dbg_qkvt", [6, 128, P * N], bf16, kind="ExternalOutput")
        dbg["osb"] = nc.dram_tensor("dbg_osb", [64, H * 2048], bf16, kind="ExternalOutput")
        dbg["ot"] = nc.dram_tensor("dbg_ot", [2, 128, P * N], bf16, kind="ExternalOutput")

    with tile.TileContext(nc) as tc:
        with (
            tc.tile_pool(name="consts", bufs=1) as consts,
            tc.tile_pool(name="xin", bufs=xin_bufs) as xin_pool,
            tc.tile_pool(name="xt", bufs=xt_bufs) as xt_pool,
            tc.tile_pool(name="qkvt", bufs=1) as qkvt_pool,
            tc.tile_pool(name="attn", bufs=attn_bufs) as attn_pool,
            tc.tile_pool(name="osb", bufs=1) as osb_pool,
            tc.tile_pool(name="ot", bufs=ot_bufs) as ot_pool,
            tc.tile_pool(name="yout", bufs=y_bufs) as y_pool,
            tc.tile_pool(name="ps_bank", bufs=bank_bufs, space="PSUM") as ps_bank,
            tc.tile_pool(name="ps_big", bufs=1, space="PSUM") as ps_big,
            tc.tile_pool(name="ps_small", bufs=small_bufs, space="PSUM") as ps_small,
        ):
            # ---------- constants ----------
            wq_sb = []
            for c0 in range(2):
                tf = consts.tile([128, 3 * C], fp32, name=f"wqf{c0}", tag=f"wqf{c0}")
                nc.sync.dma_start(out=tf, in_=wq_t.ap()[c0 * 128:(c0 + 1) * 128, :])
                t = consts.tile([128, 3 * C], bf16, name=f"wq{c0}", tag=f"wq{c0}")
                nc.any.tensor_copy(out=t, in_=tf)
                wq_sb.append(t)
            wp_bf = []
            for c0 in range(2):
                tf = consts.tile([128, C], fp32, name=f"wpf{c0}", tag=f"wpf{c0}")
                nc.sync.dma_start(out=tf, in_=wp_t.ap()[c0 * 128:(c0 + 1) * 128, :])
                tb = consts.tile([128, C], bf16, name=f"wpb{c0}", tag=f"wpb{c0}")
                nc.any.tensor_copy(out=tb, in_=tf)
                wp_bf.append(tb)
            b_bcast = consts.tile([128, C], fp32, name="b_bcast", tag="b_bcast")
            bp_b = bass.AP(tensor=bp_t, offset=0, ap=[[0, 128], [1, C]])
            nc.sync.dma_start(out=b_bcast, in_=bp_b)
            ident_b = consts.tile([128, 128], bf16, name="ident_b", tag="ident_b")
            make_identity(nc, ident_b[:])

            for b in range(B_LOC):
                # ---- S1+S2: load rows, transpose to X^T [c, t(raster)] ----
                xt_sb = [
                    xt_pool.tile([128, P * N], bf16, name=f"xt{c0}_{b}", tag=f"xt{c0}")
                    for c0 in range(2)
                ]
                for k in range(8):          # row groups of 8
                    ps_xt = [
                        ps_bank.tile([128, 512], fp32, name=f"ps_xt{cch}", tag="bank")
                        for cch in range(2)
                    ]
                    for rp in range(4):     # row pairs within group
                        r0 = k * 8 + rp * 2
                        # [64 w, (row_local, c)]: partition = w (single stride)
                        xtf = xin_pool.tile([64, 2 * C], fp32, name=f"xf_{b}_{k}_{rp}", tag="xf")
                        src = bass.AP(
                            tensor=x_t,
                            offset=(b * 4096 + r0 * 64) * C,
                            ap=[[C, 64], [64 * C, 2], [1, C]],
                        )
                        nc.sync.dma_start(out=xtf, in_=src)
                        xtile = xin_pool.tile([64, 2 * C], bf16, name=f"x_{b}_{k}_{rp}", tag="x")
                        nc.any.tensor_copy(out=xtile, in_=xtf)
                        for rl in range(2):
                            for cch in range(2):
                                nc.tensor.matmul(
                                    ps_xt[cch][:, (rp * 2 + rl) * 64:(rp * 2 + rl) * 64 + 64],
                                    xtile[:, rl * 256 + cch * 128:rl * 256 + cch * 128 + 128],
                                    ident_b[:64, :64],
                                    start=True, stop=True,
                                )
                    for cch in range(2):
                        nc.any.tensor_copy(
                            out=xt_sb[cch][:, k * 512:(k + 1) * 512],
                            in_=ps_xt[cch],
                        )

                # ---- S3: QKV^T = w_qkv.T @ X -> 6 x [128, 4096] bf16 ----
                qkvt = [
                    qkvt_pool.tile([128, P * N], bf16, name=f"qk{i}_{b}", tag=f"qk{i}")
                    for i in range(6)
                ]
                # p-major view [c, gi, pi, gj, pj] of the p-major buffer
                # qkvt free layout: p*64+n = pi*512 + pj*64 + gi*8 + gj
                qkvt_pm = [
                    q.rearrange("c (pi pj gi gj) -> c gi pi gj pj",
                                pi=8, pj=8, gi=8, gj=8)
                    for q in qkvt
                ]
                for i in range(6):
                    for tr in range(8):   # tr == gi (raster row group)
                        ps_q = ps_bank.tile([128, 512], fp32, name="ps_q", tag="bank")
                        for c0 in range(2):
                            nc.tensor.matmul(
                                ps_q,
                                wq_sb[c0][:, i * 128:(i + 1) * 128],
                                xt_sb[c0][:, tr * 512:(tr + 1) * 512],
                                start=(c0 == 0),
                                stop=(c0 == 1),
                            )
                        # psum free (raster in row-group tr): (pi, gj, pj)
                        nc.any.tensor_copy(
                            out=qkvt_pm[i][:, tr], in_=ps_q.rearrange(
                                "c (pi gj pj) -> c pi gj pj", pi=8, gj=8, pj=8)
                        )

                def chunk(m):
                    # [128 f, 64 n] contiguous in the p-major buffer
                    p = m // 6
                    return qkvt[m % 6][:, p * 64:(p + 1) * 64]

                if debug_dump and b == 0:
                    for c0 in range(2):
                        nc.gpsimd.dma_start(out=dbg["xt"].ap()[c0], in_=xt_sb[c0])
                    for i in range(6):
                        nc.gpsimd.dma_start(out=dbg["qkvt"].ap()[i], in_=qkvt[i])

                # ---- S4: attention ----
                o_sb = osb_pool.tile([64, H * 2048], bf16, name=f"osb_{b}", tag="osb")
                for hh in range(H):
                    ps_l = ps_small.tile([64, 64], fp32, name="ps_l", tag="small")
                    for j in range(16):
                        nc.tensor.matmul(
                            ps_l,
                            chunk(hh * 16 + j),
                            chunk(128 + hh * 16 + j),
                            start=(j == 0),
                            stop=(j == 15),
                        )
                    nm = attn_pool.tile([64, 1], fp32, name="nm", tag="nm")
                    nc.vector.reduce_max(
                        nm, ps_l, axis=mybir.AxisListType.X, negate=True
                    )
                    nms = attn_pool.tile([64, 1], fp32, name="nms", tag="nms")
                    nc.vector.tensor_scalar_mul(nms, nm, SCALE)
                    pe = attn_pool.tile([64, 64], fp32, name="pe", tag="pe")
                    nc.scalar.activation(
                        pe, ps_l, mybir.ActivationFunctionType.Exp,
                        bias=nms, scale=SCALE,
                    )
                    s_sum = attn_pool.tile([64, 1], fp32, name="s_sum", tag="s_sum")
                    nc.vector.reduce_sum(s_sum, pe, axis=mybir.AxisListType.X)
                    r_sum = attn_pool.tile([64, 1], fp32, name="r_sum", tag="r_sum")
                    nc.vector.reciprocal(r_sum, s_sum)
                    attn_bf = attn_pool.tile([64, 64], bf16, name="attn_bf", tag="attn_bf")
                    nc.vector.tensor_scalar_mul(attn_bf, pe, r_sum)
                    # attnT [n2, n1]
                    ps_at = ps_small.tile([64, 64], fp32, name="ps_at", tag="small")
                    nc.tensor.matmul(ps_at, attn_bf, ident_b[:64, :64],
                                     start=True, stop=True)
                    attnT = attn_pool.tile([64, 64], bf16, name="attnT", tag="attnT")
                    nc.any.tensor_copy(out=attnT, in_=ps_at)
                    # V_tok [n2, 2048]
                    v_tok = attn_pool.tile([64, 2048], bf16, name="v_tok", tag="v_tok")
                    for vb in range(4):
                        ps_v = ps_bank.tile([64, 512], fp32, name="ps_v", tag="bank")
                        for q in range(4):
                            jj = vb * 4 + q
                            nc.tensor.matmul(
                                ps_v[:, q * 128:(q + 1) * 128],
                                chunk(256 + hh * 16 + jj),
                                ident_b,
                                start=True, stop=True,
                            )
                        nc.any.tensor_copy(
                            out=v_tok[:, vb * 512:(vb + 1) * 512], in_=ps_v
                        )
                    # O = attn @ V_tok  [n1, 2048]
                    ps_o = ps_big.tile([64, 2048], fp32, name="ps_o", tag="big")
                    for ob in range(4):
                        nc.tensor.matmul(
                            ps_o[:, ob * 512:(ob + 1) * 512],
                            attnT,
                            v_tok[:, ob * 512:(ob + 1) * 512],
                            start=True,
                            stop=True,
                        )
                    nc.any.tensor_copy(
                        out=o_sb[:, hh * 2048:(hh + 1) * 2048], in_=ps_o
                    )

                # ---- S5: O^T (raster t), proj, store ----
                ot_sb = [
                    ot_pool.tile([128, P * N], bf16, name=f"ot{c0}_{b}", tag=f"ot{c0}")
                    for c0 in range(2)
                ]
                # raster view: [c, pi, pj, gi, gj]
                ot_r = [
                    t.rearrange("c (gi pi gj pj) -> c pi pj gi gj",
                                gi=8, pi=8, gj=8, pj=8)
                    for t in ot_sb
                ]
                for cch in range(2):
                    for pg in range(8):  # p3 group = fixed pi3
                        ps_ot = ps_bank.tile([128, 512], fp32, name="ps_ot", tag="bank")
                        for s in range(8):
                            p3 = pg * 8 + s
                            nc.tensor.matmul(
                                ps_ot[:, s * 64:(s + 1) * 64],
                                o_sb[:, p3 * 256 + cch * 128:p3 * 256 + cch * 128 + 128],
                                ident_b[:64, :64],
                                start=True, stop=True,
                            )
                        # psum free order (pj3=s, gi, gj) == ot_r dims (pj, gi, gj)
                        nc.any.tensor_copy(
                            out=ot_r[cch][:, pg], in_=ps_ot
                        )
                if debug_dump and b == 0:
                    nc.gpsimd.dma_start(out=dbg["osb"].ap(), in_=o_sb)
                    for c0 in range(2):
                        nc.gpsimd.dma_start(out=dbg["ot"].ap()[c0], in_=ot_sb[c0])
                for tt in range(32):  # t'-tile: rows {2tt, 2tt+1}
                    ps_y = ps_bank.tile([128, 512], fp32, name="ps_y", tag="bank")
                    for c0 in range(2):
                        nc.tensor.matmul(
                            ps_y[:, :C],
                            ot_sb[c0][:, tt * 128:(tt + 1) * 128],
                            wp_bf[c0],
                            start=(c0 == 0),
                            stop=(c0 == 1),
                        )
                    y_sb = y_pool.tile([128, C], fp32, name="y_sb", tag="y_sb")
                    nc.vector.tensor_tensor(
                        out=y_sb, in0=ps_y[:, :C], in1=b_bcast,
                        op=mybir.AluOpType.add,
                    )
                    nc.gpsimd.dma_start(
                        out=out_flat[b, tt * 128:(tt + 1) * 128, :], in_=y_sb
                    )

    nc.compile()
    return nc


_NC_CACHE = None


def kernel(x, w_qkv, w_proj, b_proj):
    global _NC_CACHE
    from concourse import bass_utils

    x = np.ascontiguousarray(np.asarray(x, dtype=np.float32))
    w_qkv = np.ascontiguousarray(np.asarray(w_qkv, dtype=np.float32))
    w_proj = np.ascontiguousarray(np.asarray(w_proj, dtype=np.float32))
    b_proj = np.ascontiguousarray(np.asarray(b_proj, dtype=np.float32))

    if _NC_CACHE is None:
        _NC_CACHE = _build_nc()
    nc = _NC_CACHE

    in_maps = []
    for c in range(N_CORES):
        in_maps.append({
            "x": x[c * B_LOC:(c + 1) * B_LOC],
            "w_qkv": w_qkv,
            "w_proj": w_proj,
            "b_proj": b_proj,
        })
    res = bass_utils.run_bass_kernel_spmd(nc, in_maps, list(range(N_CORES)))
    out = np.concatenate([r["out"] for r in res.results], axis=0)
    return out.astype(np.float32)


if __name__ == "__main__":
    nc = _build_nc()
    print("built ok")

